# revision 38
# baseline (speedup 1.0000x reference)
# AttnBlock (GroupNorm + single-head self-attention + proj + residual) on 8 NeuronCores.
#
# Sharding: core = 2*b + ih  (b in 0..3 batch, ih in 0..1 query-half).
# Each core gets the full x[b] (needed for GN stats and full-j K/V), computes
# K/V over all 4096 positions, and Q/attention/proj for its 2048 query columns.
# No cross-core communication; host gathers the 8 [512, 2048] output shards.
#
# All heavy matmuls run as float32r (full PE rate at N>=256, fp32 storage).
# Attention scores are computed directly in S^T[j, i] layout (lhsT=k, rhs=q) so
# no on-chip transposes are needed anywhere; softmax uses no max subtraction
# (logits are ~N(0,1) by construction: normalized activations x 1/sqrt(C)
# weights x 1/sqrt(C) attn scale; |s| < ~6 << fp32 exp range).
# The softmax denominator is accumulated per j-chunk on DVE and reduced
# across partitions with a ones-column matmul; 1/l is applied after the
# projection matmul (diag scaling commutes through wp on the right).

import numpy as np

C = 512
N = 4096
B = 4
P = 128
CCH = C // P          # 4 channel chunks
IH = N // 2           # 2048 query columns per core
JT = 512              # phase-1 j tile
ITILE = 256           # phase-2 i tile (psum free dim; >=256 keeps f32r fast)
NIT = IH // ITILE     # 8 i tiles
NJC = N // P          # 32 j chunks
EPS = 1e-5
ATT_SCALE = 1.0 / float(np.sqrt(C))

LAST_EXEC_NS = None
_CACHE = {}


def _build_nc():
    import concourse.bass as bass
    import concourse.bacc as bacc
    import concourse.tile as tile
    from concourse import mybir

    f32 = mybir.dt.float32
    f32r = mybir.dt.float32r
    ALU = mybir.AluOpType
    ACT = mybir.ActivationFunctionType

    # Bacc: its compile() pipeline splits multi-wait DMAs into
    # InstEventSemaphore chains (HW allows 1 sync wait per DMA).
    nc = bacc.Bacc("TRN2", target_bir_lowering=False)

    x_h = nc.dram_tensor("x", [C, N], f32, kind="ExternalInput")
    wqT_h = nc.dram_tensor("wqT", [C, C], f32r, kind="ExternalInput")
    wkT_h = nc.dram_tensor("wkT", [C, C], f32r, kind="ExternalInput")
    wvT_h = nc.dram_tensor("wvT", [C, C], f32r, kind="ExternalInput")
    wpT_h = nc.dram_tensor("wpT", [C, C], f32r, kind="ExternalInput")
    gam_h = nc.dram_tensor("gamma", [C], f32, kind="ExternalInput")
    bet_h = nc.dram_tensor("beta", [C], f32, kind="ExternalInput")
    bq_h = nc.dram_tensor("bq", [C], f32, kind="ExternalInput")
    bk_h = nc.dram_tensor("bk", [C], f32, kind="ExternalInput")
    bv_h = nc.dram_tensor("bv", [C], f32, kind="ExternalInput")
    bp_h = nc.dram_tensor("bp", [C], f32, kind="ExternalInput")
    y_h = nc.dram_tensor("y", [C, IH], f32, kind="ExternalOutput")

    q_dram = nc.dram_tensor("q_scratch", [CCH, P, IH], f32r)
    xr_dram = nc.dram_tensor("xr_scratch", [CCH, P, IH], f32r)

    x3 = x_h[:, :].rearrange("(c p) n -> p c n", p=P)        # [128, 4, 4096]
    y3 = y_h[:, :].rearrange("(o p) n -> p o n", p=P)        # [128, 4, 2048]

    def chan_vec(h):
        # [C] dram -> [128, CCH] sbuf view (partition p, chunk c) = elem c*128+p
        return h[:].rearrange("(c p) -> p c", p=P)

    with tile.TileContext(nc) as tc:
        ctx_lp = nc.allow_low_precision(
            "float32r tiles are fp32-width storage; rounding only at PE"
        )
        ctx_lp.__enter__()
        with (
            tc.tile_pool(name="persist", bufs=1) as pers,
            tc.tile_pool(name="wpool", bufs=3) as wpool,
            tc.tile_pool(name="pstream", bufs=2) as pstream,
            tc.tile_pool(name="ps", bufs=7, space="PSUM") as ps,
        ):
            # ---- persistent tensors ----
            k_sb = pers.tile([P, CCH, N], f32r, tag="k")        # 64 KB/part
            vT_sb = pers.tile([P, NJC, C], f32r, tag="vT")      # 64 KB/part
            gam_t = pers.tile([P, CCH], f32, tag="gam")
            bet_t = pers.tile([P, CCH], f32, tag="bet")
            bq_t = pers.tile([P, CCH], f32, tag="bq")
            bk_t = pers.tile([P, CCH], f32, tag="bk")
            bv_t = pers.tile([P, CCH], f32, tag="bv")
            bp_t = pers.tile([P, CCH], f32, tag="bp")
            scale_c = pers.tile([P, CCH], f32, tag="scale_c")  # rstd*gamma per chan
            shift_c = pers.tile([P, CCH], f32, tag="shift_c")  # beta - mu*scale
            ones_col = pers.tile([P, 1], f32, tag="ones_col")
            ones_row = pers.tile([1, P], f32, tag="ones_row")

            nc.sync.dma_start(out=gam_t, in_=chan_vec(gam_h))
            nc.sync.dma_start(out=bet_t, in_=chan_vec(bet_h))
            nc.sync.dma_start(out=bq_t, in_=chan_vec(bq_h))
            nc.sync.dma_start(out=bk_t, in_=chan_vec(bk_h))
            nc.sync.dma_start(out=bv_t, in_=chan_vec(bv_h))
            nc.sync.dma_start(out=bp_t, in_=chan_vec(bp_h))
            nc.vector.memset(ones_col, 1.0)
            nc.vector.memset(ones_row, 1.0)
            wkT = wpool.tile([P, CCH, C], f32r, tag="w")
            wvT = wpool.tile([P, CCH, C], f32r, tag="w")
            wqT = wpool.tile([P, CCH, C], f32r, tag="w")

            # ========== Phase 0+1: stats, then K/V/Q in one scope ==========
            # The stats pass and compute pass share the x-tile slots; phase 1
            # visits j-tiles 7,6 first (still resident from the stats sweep)
            # so PE starts as soon as the affine coefficients exist.
            with tc.tile_pool(name="p1", bufs=2) as p1:
                p2 = p1
                ind64 = p1.tile([P, 2], f32, tag="ind64", bufs=1)
                nc.vector.memset(ind64, 0.0)
                nc.vector.memset(ind64[0:64, 0:1], 1.0 / 64.0)
                nc.vector.memset(ind64[64:128, 1:2], 1.0 / 64.0)
                # bcT[g, p] = 1.0 where p//64 == g (engine writes must start
                # at 32-aligned partitions, hence affine selects)
                bcT = p1.tile([2, P], f32, tag="bcT", bufs=1)
                nc.gpsimd.memset(bcT, 1.0)
                nc.gpsimd.affine_select(
                    out=bcT, in_=bcT, compare_op=ALU.is_ge, fill=0.0,
                    base=0, pattern=[[1, P]], channel_multiplier=-64,
                )
                nc.gpsimd.affine_select(
                    out=bcT, in_=bcT, compare_op=ALU.is_ge, fill=0.0,
                    base=63, pattern=[[-1, P]], channel_multiplier=64,
                )
                eps2 = p1.tile([2, 1], f32, tag="eps2", bufs=1)
                nc.vector.memset(eps2, EPS)

                stats = p1.tile([P, CCH, N // JT, 6], f32, tag="stats", bufs=1)
                xtiles = {}
                for jt in range(N // JT):
                    xjs = p1.tile([P, CCH, JT], f32, tag="xjs")
                    nc.sync.dma_start(
                        out=xjs, in_=x3[:, :, jt * JT:(jt + 1) * JT]
                    )
                    xtiles[jt] = xjs
                    for c in range(CCH):
                        nc.vector.bn_stats(
                            out=stats[:, c, jt, :], in_=xjs[:, c, :]
                        )
                # weights stream while the stats pipeline finishes (k first:
                # phase 1 starts with k/v on the resident j-tiles)
                nc.sync.dma_start(
                    out=wkT, in_=wkT_h[:, :].rearrange("(c p) o -> p c o", p=P)
                )
                nc.sync.dma_start(
                    out=wvT, in_=wvT_h[:, :].rearrange("(c p) o -> p c o", p=P)
                )
                nc.sync.dma_start(
                    out=wqT, in_=wqT_h[:, :].rearrange("(c p) o -> p c o", p=P)
                )

                mv = p1.tile([P, CCH, 2], f32, tag="mv", bufs=1)
                st8 = p1.tile([P, CCH, 2], f32, tag="st8", bufs=1)
                m2 = p1.tile([P, 1], f32, tag="m2", bufs=1)
                for c in range(CCH):
                    nc.vector.bn_aggr(out=mv[:, c, :], in_=stats[:, c, :, :])
                    nc.vector.tensor_copy(out=st8[:, c, 0:1], in_=mv[:, c, 0:1])
                    nc.vector.tensor_mul(m2, mv[:, c, 0:1], mv[:, c, 0:1])
                    nc.vector.tensor_add(st8[:, c, 1:2], mv[:, c, 1:2], m2)
                gsp = ps.tile([2, CCH, 2], f32, tag="ps")
                nc.tensor.matmul(
                    gsp, ind64, st8.rearrange("p c t -> p (c t)"),
                    start=True, stop=True,
                )
                gs = p1.tile([2, CCH, 2], f32, tag="gs", bufs=1)
                nc.vector.tensor_copy(out=gs, in_=gsp)
                musq = p1.tile([2, CCH], f32, tag="musq", bufs=1)
                varg = p1.tile([2, CCH], f32, tag="varg", bufs=1)
                nc.vector.tensor_mul(musq, gs[:, :, 0], gs[:, :, 0])
                nc.vector.tensor_tensor(
                    out=varg, in0=gs[:, :, 1], in1=musq, op=ALU.subtract
                )
                nc.scalar.activation(
                    out=varg, in_=varg, func=ACT.Sqrt, bias=eps2
                )
                nc.vector.reciprocal(out=varg, in_=varg)
                ms = p1.tile([2, 2 * CCH], f32, tag="ms", bufs=1)
                nc.vector.tensor_copy(out=ms[:, 0:CCH], in_=gs[:, :, 0])
                nc.vector.tensor_copy(out=ms[:, CCH:2 * CCH], in_=varg)
                bcp = ps.tile([P, 2 * CCH], f32, tag="ps")
                nc.tensor.matmul(bcp, bcT, ms, start=True, stop=True)
                mcrc = p1.tile([P, 2 * CCH], f32, tag="mcrc", bufs=1)
                nc.vector.tensor_copy(out=mcrc, in_=bcp)
                tmp4 = p1.tile([P, CCH], f32, tag="tmp4", bufs=1)
                nc.vector.tensor_mul(scale_c, mcrc[:, CCH:2 * CCH], gam_t)
                nc.vector.tensor_mul(tmp4, mcrc[:, 0:CCH], scale_c)
                nc.vector.tensor_tensor(
                    out=shift_c, in0=bet_t, in1=tmp4, op=ALU.subtract
                )

                prefetched = {}
                q4 = q_dram[:, :, :].rearrange("o p n -> p o n")
                xr4 = xr_dram[:, :, :].rearrange("c p n -> p c n")
                for jt in [7, 6, 0, 1, 2, 3, 4, 5]:
                    if jt in (7, 6):
                        xjs = xtiles[jt]  # still resident from the stats pass
                    else:
                        xjs = p1.tile([P, CCH, JT], f32, tag="xjs")
                        nc.sync.dma_start(
                            out=xjs, in_=x3[:, :, jt * JT:(jt + 1) * JT]
                        )
                    xn = p1.tile([P, CCH, JT], f32r, tag="xn")
                    for c in range(CCH):
                        nc.vector.tensor_scalar(
                            out=xn[:, c, :], in0=xjs[:, c, :],
                            scalar1=scale_c[:, c:c + 1],
                            scalar2=shift_c[:, c:c + 1],
                            op0=ALU.mult, op1=ALU.add,
                        )
                    for o in range(CCH):
                        pk = ps.tile([P, JT], f32, tag="ps")
                        for c in range(CCH):
                            nc.tensor.matmul(
                                pk,
                                wkT[:, c, o * P:(o + 1) * P],
                                xn[:, c, :],
                                start=(c == 0), stop=(c == CCH - 1),
                            )
                        nc.vector.tensor_scalar(
                            out=k_sb[:, o, jt * JT:(jt + 1) * JT], in0=pk,
                            scalar1=bk_t[:, o:o + 1], scalar2=None,
                            op0=ALU.add,
                        )
                    for js in range(JT // P):
                        pv = ps.tile([P, C], f32, tag="ps")
                        for c in range(CCH):
                            nc.tensor.matmul(
                                pv,
                                xn[:, c, js * P:(js + 1) * P],
                                wvT[:, c, :],
                                start=(c == 0), stop=(c == CCH - 1),
                            )
                        jc = jt * (JT // P) + js
                        nc.vector.tensor_copy(out=vT_sb[:, jc, :], in_=pv)
                    if jt < IH // JT:
                        it = jt
                        # query i-tile: q matmuls + residual store share xn
                        nc.sync.dma_start(
                            out=xr_dram[:, :, it * JT:(it + 1) * JT].rearrange(
                                "c p i -> p c i"
                            ),
                            in_=xn,
                        )
                        for o in range(CCH):
                            pq = ps.tile([P, JT], f32, tag="ps")
                            for c in range(CCH):
                                nc.tensor.matmul(
                                    pq,
                                    wqT[:, c, o * P:(o + 1) * P],
                                    xn[:, c, :],
                                    start=(c == 0), stop=(c == CCH - 1),
                                )
                            qt = p2.tile([P, JT], f32r, tag="qt", bufs=1)
                            nc.vector.tensor_scalar(
                                out=qt, in0=pq,
                                scalar1=bq_t[:, o:o + 1], scalar2=None,
                                op0=ALU.add,
                            )
                            nc.sync.dma_start(
                                out=q_dram[o, :, it * JT:(it + 1) * JT],
                                in_=qt,
                            )
                        if it == 0:
                            qt2 = pstream.tile(
                                [P, CCH, ITILE], f32r, tag="qt2"
                            )
                            nc.sync.dma_start(out=qt2, in_=q4[:, :, 0:ITILE])
                            xr0 = pstream.tile(
                                [P, CCH, ITILE], f32r, tag="xr", bufs=1
                            )
                            nc.sync.dma_start(out=xr0, in_=xr4[:, :, 0:ITILE])
                            prefetched[0] = (qt2, xr0)

            # ================= Phase 2: attention + proj =================
            with tc.tile_pool(name="p3", bufs=2) as p3:
                wpT = wpool.tile([P, CCH, C], f32r, tag="w")
                nc.sync.dma_start(
                    out=wpT, in_=wpT_h[:, :].rearrange("(c p) o -> p c o", p=P)
                )
                # v-bias folds to a constant output bias: y += wp@bv + bp
                # (attention rows sum to 1 after the linv scaling).
                bias2 = pstream.tile([P, CCH], f32, tag="bias2", bufs=1)
                for oc in range(CCH):
                    pbv = ps.tile([P, 1], f32, tag="ps")
                    for cc in range(CCH):
                        nc.tensor.matmul(
                            pbv,
                            wpT[:, cc, oc * P:(oc + 1) * P].bitcast(f32),
                            bv_t[:, cc:cc + 1],
                            start=(cc == 0), stop=(cc == CCH - 1),
                        )
                    nc.vector.tensor_scalar(
                        out=bias2[:, oc:oc + 1], in0=pbv,
                        scalar1=bp_t[:, oc:oc + 1], scalar2=None, op0=ALU.add,
                    )
                for t in range(NIT):
                    isl = slice(t * ITILE, (t + 1) * ITILE)
                    if t in prefetched:
                        qt2, xr = prefetched[t]
                    else:
                        qt2 = pstream.tile([P, CCH, ITILE], f32r, tag="qt2")
                        nc.sync.dma_start(out=qt2, in_=q4[:, :, isl])
                        xr = pstream.tile([P, CCH, ITILE], f32r, tag="xr", bufs=1)
                        nc.sync.dma_start(out=xr, in_=xr4[:, :, isl])
                    PT = p3.tile([P, NJC, ITILE], f32r, tag="PT", bufs=1)
                    # two alternating partial softmax-denominator
                    # accumulators: a single serial 32-add DVE chain would lag
                    # the exps and stall PE at the pl matmul.
                    lp4 = p3.tile([P, 2, ITILE], f32, tag="lp4", bufs=1)
                    for jc in range(NJC):
                        pS = ps.tile([P, ITILE], f32, tag="ps")
                        for c in range(CCH):
                            nc.tensor.matmul(
                                pS,
                                k_sb[:, c, jc * P:(jc + 1) * P],
                                qt2[:, c, :],
                                start=(c == 0), stop=(c == CCH - 1),
                            )
                        nc.scalar.activation(
                            out=PT[:, jc, :], in_=pS, func=ACT.Exp,
                            scale=ATT_SCALE,
                        )
                        acc = lp4[:, jc % 2, :]
                        if jc < 2:
                            nc.vector.tensor_copy(out=acc, in_=PT[:, jc, :])
                        else:
                            nc.vector.tensor_add(acc, acc, PT[:, jc, :])

                    # PV before the l-reduction matmuls: PE stays busy while
                    # DVE finishes the partial sums.
                    ao = p3.tile([P, CCH, ITILE], f32r, tag="ao", bufs=1)
                    for cc in range(CCH):
                        pPV = ps.tile([P, ITILE], f32, tag="ps")
                        for jc in range(NJC):
                            nc.tensor.matmul(
                                pPV,
                                vT_sb[:, jc, cc * P:(cc + 1) * P],
                                PT[:, jc, :],
                                start=(jc == 0), stop=(jc == NJC - 1),
                            )
                        nc.vector.tensor_copy(out=ao[:, cc, :], in_=pPV)

                    nc.vector.tensor_add(lp4[:, 0, :], lp4[:, 0, :], lp4[:, 1, :])
                    pl = ps.tile([1, ITILE], f32, tag="ps")
                    nc.tensor.matmul(
                        pl, ones_col, lp4[:, 0, :],
                        start=True, stop=True,
                    )
                    linv = pstream.tile([1, ITILE], f32, tag="linv", bufs=1)
                    nc.vector.reciprocal(out=linv, in_=pl)
                    pb = ps.tile([P, ITILE], f32, tag="ps")
                    nc.tensor.matmul(
                        pb, ones_row, linv,
                        start=True, stop=True,
                    )
                    lb = p3.tile([P, ITILE], f32, tag="lb", bufs=1)
                    nc.vector.tensor_copy(out=lb, in_=pb)
                    for oc in range(CCH):
                        pY = ps.tile([P, ITILE], f32, tag="ps")
                        for cc in range(CCH):
                            nc.tensor.matmul(
                                pY,
                                wpT[:, cc, oc * P:(oc + 1) * P],
                                ao[:, cc, :],
                                start=(cc == 0), stop=(cc == CCH - 1),
                            )
                        yt = p3.tile([P, ITILE], f32, tag="yt")
                        nc.vector.tensor_mul(yt, pY, lb)
                        nc.vector.tensor_scalar(
                            out=yt, in0=yt, scalar1=bias2[:, oc:oc + 1],
                            scalar2=None, op0=ALU.add,
                        )
                        nc.vector.tensor_add(yt, yt, xr[:, oc, :])
                        nc.sync.dma_start(out=y3[:, oc, isl], in_=yt)
    nc.finalize()
    return nc


def _make_in_maps(x, gn_gamma, gn_beta, wq, bq, wk, bk, wv, bv, wp, bp):
    x = np.asarray(x, dtype=np.float32)
    xr = np.ascontiguousarray(x.reshape(B, C, N))
    wqT = np.ascontiguousarray(np.asarray(wq, np.float32).T)
    wkT = np.ascontiguousarray(np.asarray(wk, np.float32).T)
    wvT = np.ascontiguousarray(np.asarray(wv, np.float32).T)
    wpT = np.ascontiguousarray(np.asarray(wp, np.float32).T)
    shared = {
        "wqT": wqT, "wkT": wkT, "wvT": wvT, "wpT": wpT,
        "gamma": np.ascontiguousarray(np.asarray(gn_gamma, np.float32)),
        "beta": np.ascontiguousarray(np.asarray(gn_beta, np.float32)),
        "bq": np.ascontiguousarray(np.asarray(bq, np.float32)),
        "bk": np.ascontiguousarray(np.asarray(bk, np.float32)),
        "bv": np.ascontiguousarray(np.asarray(bv, np.float32)),
        "bp": np.ascontiguousarray(np.asarray(bp, np.float32)),
    }
    in_maps = []
    for core in range(8):
        b, ih = core // 2, core % 2
        # rotate spatial columns so this core's query half is always 0..IH-1
        # (GroupNorm and attention are permutation-invariant over positions)
        xrot = xr[b] if ih == 0 else np.concatenate(
            [xr[b][:, IH:], xr[b][:, :IH]], axis=1
        )
        in_maps.append({"x": np.ascontiguousarray(xrot), **shared})

    return in_maps


def _gather(results):
    out = np.empty((B, C, N), np.float32)
    for core in range(8):
        b, ih = core // 2, core % 2
        out[b][:, ih * IH:(ih + 1) * IH] = results[core]["y"]
    return out.reshape(B, C, 64, 64)


def kernel(**inputs):
    global LAST_EXEC_NS
    from concourse.bass_utils import run_bass_kernel_spmd

    if "nc" not in _CACHE:
        _CACHE["nc"] = _build_nc()
    nc = _CACHE["nc"]
    in_maps = _make_in_maps(**inputs)
    res = run_bass_kernel_spmd(nc, in_maps, list(range(8)))
    LAST_EXEC_NS = res.exec_time_ns
    return _gather(res.results)



# revision 39
# speedup vs baseline: 1.0849x; 1.0849x over previous
# AttnBlock (GroupNorm + single-head self-attention + proj + residual) on 8 NeuronCores.
#
# Sharding: core = 2*b + ih  (b in 0..3 batch, ih in 0..1 query-half).
# Each core gets the full x[b] (needed for GN stats and full-j K/V), computes
# K/V over all 4096 positions, and Q/attention/proj for its 2048 query columns.
# No cross-core communication; host gathers the 8 [512, 2048] output shards.
#
# All heavy matmuls run as float32r (full PE rate at N>=256, fp32 storage).
# Attention scores are computed directly in S^T[j, i] layout (lhsT=k, rhs=q) so
# no on-chip transposes are needed anywhere; softmax uses no max subtraction
# (logits are ~N(0,1) by construction: normalized activations x 1/sqrt(C)
# weights x 1/sqrt(C) attn scale; |s| < ~6 << fp32 exp range).
# The softmax denominator is accumulated per j-chunk on DVE and reduced
# across partitions with a ones-column matmul; 1/l is applied after the
# projection matmul (diag scaling commutes through wp on the right).

import numpy as np

C = 512
N = 4096
B = 4
P = 128
CCH = C // P          # 4 channel chunks
IH = N // 2           # 2048 query columns per core
JT = 512              # phase-1 j tile
ITILE = 256           # phase-2 i tile (psum free dim; >=256 keeps f32r fast)
NIT = IH // ITILE     # 8 i tiles
NJC = N // P          # 32 j chunks
EPS = 1e-5
ATT_SCALE = 1.0 / float(np.sqrt(C))

LAST_EXEC_NS = None
_CACHE = {}


def _build_nc():
    import concourse.bass as bass
    import concourse.bacc as bacc
    import concourse.tile as tile
    from concourse import mybir

    f32 = mybir.dt.float32
    f32r = mybir.dt.float32r
    ALU = mybir.AluOpType
    ACT = mybir.ActivationFunctionType

    # Bacc: its compile() pipeline splits multi-wait DMAs into
    # InstEventSemaphore chains (HW allows 1 sync wait per DMA).
    nc = bacc.Bacc("TRN2", target_bir_lowering=False)

    x_h = nc.dram_tensor("x", [C, N], f32, kind="ExternalInput")
    wqT_h = nc.dram_tensor("wqT", [C, C], f32r, kind="ExternalInput")
    wkT_h = nc.dram_tensor("wkT", [C, C], f32r, kind="ExternalInput")
    wvT_h = nc.dram_tensor("wvT", [C, C], f32r, kind="ExternalInput")
    wpT_h = nc.dram_tensor("wpT", [C, C], f32r, kind="ExternalInput")
    gam_h = nc.dram_tensor("gamma", [C], f32, kind="ExternalInput")
    bet_h = nc.dram_tensor("beta", [C], f32, kind="ExternalInput")
    bq_h = nc.dram_tensor("bq", [C], f32, kind="ExternalInput")
    bk_h = nc.dram_tensor("bk", [C], f32, kind="ExternalInput")
    bv_h = nc.dram_tensor("bv", [C], f32, kind="ExternalInput")
    bp_h = nc.dram_tensor("bp", [C], f32, kind="ExternalInput")
    y_h = nc.dram_tensor("y", [C, IH], f32, kind="ExternalOutput")

    q_dram = nc.dram_tensor("q_scratch", [CCH, P, IH], f32r)
    xr_dram = nc.dram_tensor("xr_scratch", [CCH, P, IH], f32r)

    x3 = x_h[:, :].rearrange("(c p) n -> p c n", p=P)        # [128, 4, 4096]
    y3 = y_h[:, :].rearrange("(o p) n -> p o n", p=P)        # [128, 4, 2048]

    def chan_vec(h):
        # [C] dram -> [128, CCH] sbuf view (partition p, chunk c) = elem c*128+p
        return h[:].rearrange("(c p) -> p c", p=P)

    with tile.TileContext(nc) as tc:
        ctx_lp = nc.allow_low_precision(
            "float32r tiles are fp32-width storage; rounding only at PE"
        )
        ctx_lp.__enter__()
        with (
            tc.tile_pool(name="persist", bufs=1) as pers,
            tc.tile_pool(name="wpool", bufs=3) as wpool,
            tc.tile_pool(name="pstream", bufs=2) as pstream,
            tc.tile_pool(name="ps", bufs=7, space="PSUM") as ps,
        ):
            # ---- persistent tensors ----
            k_sb = pers.tile([P, CCH, N], f32r, tag="k")        # 64 KB/part
            vT_sb = pers.tile([P, NJC, C], f32r, tag="vT")      # 64 KB/part
            gam_t = pers.tile([P, CCH], f32, tag="gam")
            bet_t = pers.tile([P, CCH], f32, tag="bet")
            bq_t = pers.tile([P, CCH], f32, tag="bq")
            bk_t = pers.tile([P, CCH], f32, tag="bk")
            bv_t = pers.tile([P, CCH], f32, tag="bv")
            bp_t = pers.tile([P, CCH], f32, tag="bp")
            scale_c = pers.tile([P, CCH], f32, tag="scale_c")  # rstd*gamma per chan
            shift_c = pers.tile([P, CCH], f32, tag="shift_c")  # beta - mu*scale
            ones_col = pers.tile([P, 1], f32, tag="ones_col")
            ones_row = pers.tile([1, P], f32, tag="ones_row")

            nc.vector.memset(ones_col, 1.0)
            nc.vector.memset(ones_row, 1.0)
            ones_col_r = pers.tile([P, 1], f32r, tag="ones_col_r")
            ones_row_r = pers.tile([1, P], f32r, tag="ones_row_r")
            nc.vector.tensor_copy(out=ones_col_r, in_=ones_col)
            nc.vector.tensor_copy(out=ones_row_r, in_=ones_row)
            wkT = wpool.tile([P, CCH, C], f32r, tag="w")
            wvT = wpool.tile([P, CCH, C], f32r, tag="w")
            wqT = wpool.tile([P, CCH, C], f32r, tag="w")

            # ========== Phase 0+1: stats, then K/V/Q in one scope ==========
            # The stats pass and compute pass share the x-tile slots; phase 1
            # visits j-tiles 7,6 first (still resident from the stats sweep)
            # so PE starts as soon as the affine coefficients exist.
            with tc.tile_pool(name="p1", bufs=2) as p1:
                p2 = p1
                ind64 = p1.tile([P, 2], f32, tag="ind64", bufs=1)
                nc.vector.memset(ind64, 0.0)
                nc.vector.memset(ind64[0:64, 0:1], 1.0 / 64.0)
                nc.vector.memset(ind64[64:128, 1:2], 1.0 / 64.0)
                # bcT[g, p] = 1.0 where p//64 == g (engine writes must start
                # at 32-aligned partitions, hence affine selects)
                bcT = p1.tile([2, P], f32, tag="bcT", bufs=1)
                nc.gpsimd.memset(bcT, 1.0)
                nc.gpsimd.affine_select(
                    out=bcT, in_=bcT, compare_op=ALU.is_ge, fill=0.0,
                    base=0, pattern=[[1, P]], channel_multiplier=-64,
                )
                nc.gpsimd.affine_select(
                    out=bcT, in_=bcT, compare_op=ALU.is_ge, fill=0.0,
                    base=63, pattern=[[-1, P]], channel_multiplier=64,
                )
                eps2 = p1.tile([2, 1], f32, tag="eps2", bufs=1)
                nc.vector.memset(eps2, EPS)

                stats = p1.tile([P, CCH, N // JT, 6], f32, tag="stats", bufs=1)
                xtiles = {}
                for jt in range(N // JT):
                    xjs = p1.tile([P, CCH, JT], f32, tag="xjs")
                    nc.sync.dma_start(
                        out=xjs, in_=x3[:, :, jt * JT:(jt + 1) * JT]
                    )
                    xtiles[jt] = xjs
                    for c in range(CCH):
                        nc.vector.bn_stats(
                            out=stats[:, c, jt, :], in_=xjs[:, c, :]
                        )
                # bias vectors and weights stream while the stats pipeline
                # finishes (k's weight first: phase 1 starts with k/v)
                nc.sync.dma_start(out=gam_t, in_=chan_vec(gam_h))
                nc.sync.dma_start(out=bet_t, in_=chan_vec(bet_h))
                nc.sync.dma_start(out=bq_t, in_=chan_vec(bq_h))
                nc.sync.dma_start(out=bk_t, in_=chan_vec(bk_h))
                nc.sync.dma_start(out=bv_t, in_=chan_vec(bv_h))
                nc.sync.dma_start(out=bp_t, in_=chan_vec(bp_h))
                nc.sync.dma_start(
                    out=wkT, in_=wkT_h[:, :].rearrange("(c p) o -> p c o", p=P)
                )
                nc.sync.dma_start(
                    out=wvT, in_=wvT_h[:, :].rearrange("(c p) o -> p c o", p=P)
                )
                nc.sync.dma_start(
                    out=wqT, in_=wqT_h[:, :].rearrange("(c p) o -> p c o", p=P)
                )

                mv = p1.tile([P, CCH, 2], f32, tag="mv", bufs=1)
                st8 = p1.tile([P, CCH, 2], f32, tag="st8", bufs=1)
                m2 = p1.tile([P, 1], f32, tag="m2", bufs=1)
                for c in range(CCH):
                    nc.vector.bn_aggr(out=mv[:, c, :], in_=stats[:, c, :, :])
                    nc.vector.tensor_copy(out=st8[:, c, 0:1], in_=mv[:, c, 0:1])
                    nc.vector.tensor_mul(m2, mv[:, c, 0:1], mv[:, c, 0:1])
                    nc.vector.tensor_add(st8[:, c, 1:2], mv[:, c, 1:2], m2)
                gsp = ps.tile([2, CCH, 2], f32, tag="ps")
                nc.tensor.matmul(
                    gsp, ind64, st8.rearrange("p c t -> p (c t)"),
                    start=True, stop=True,
                )
                gs = p1.tile([2, CCH, 2], f32, tag="gs", bufs=1)
                nc.vector.tensor_copy(out=gs, in_=gsp)
                musq = p1.tile([2, CCH], f32, tag="musq", bufs=1)
                varg = p1.tile([2, CCH], f32, tag="varg", bufs=1)
                nc.vector.tensor_mul(musq, gs[:, :, 0], gs[:, :, 0])
                nc.vector.tensor_tensor(
                    out=varg, in0=gs[:, :, 1], in1=musq, op=ALU.subtract
                )
                nc.scalar.activation(
                    out=varg, in_=varg, func=ACT.Sqrt, bias=eps2
                )
                nc.vector.reciprocal(out=varg, in_=varg)
                ms = p1.tile([2, 2 * CCH], f32, tag="ms", bufs=1)
                nc.vector.tensor_copy(out=ms[:, 0:CCH], in_=gs[:, :, 0])
                nc.vector.tensor_copy(out=ms[:, CCH:2 * CCH], in_=varg)
                bcp = ps.tile([P, 2 * CCH], f32, tag="ps")
                nc.tensor.matmul(bcp, bcT, ms, start=True, stop=True)
                mcrc = p1.tile([P, 2 * CCH], f32, tag="mcrc", bufs=1)
                nc.vector.tensor_copy(out=mcrc, in_=bcp)
                tmp4 = p1.tile([P, CCH], f32, tag="tmp4", bufs=1)
                nc.vector.tensor_mul(scale_c, mcrc[:, CCH:2 * CCH], gam_t)
                nc.vector.tensor_mul(tmp4, mcrc[:, 0:CCH], scale_c)
                nc.vector.tensor_tensor(
                    out=shift_c, in0=bet_t, in1=tmp4, op=ALU.subtract
                )

                prefetched = {}
                q4 = q_dram[:, :, :].rearrange("o p n -> p o n")
                xr4 = xr_dram[:, :, :].rearrange("c p n -> p c n")
                for jt in [7, 6, 0, 1, 2, 3, 4, 5]:
                    if jt in (7, 6):
                        xjs = xtiles[jt]  # still resident from the stats pass
                    else:
                        xjs = p1.tile([P, CCH, JT], f32, tag="xjs")
                        nc.sync.dma_start(
                            out=xjs, in_=x3[:, :, jt * JT:(jt + 1) * JT]
                        )
                    xn = p1.tile([P, CCH, JT], f32r, tag="xn")
                    for c in range(CCH):
                        nc.vector.tensor_scalar(
                            out=xn[:, c, :], in0=xjs[:, c, :],
                            scalar1=scale_c[:, c:c + 1],
                            scalar2=shift_c[:, c:c + 1],
                            op0=ALU.mult, op1=ALU.add,
                        )
                    for o in range(CCH):
                        pk = ps.tile([P, JT], f32, tag="ps")
                        for c in range(CCH):
                            nc.tensor.matmul(
                                pk,
                                wkT[:, c, o * P:(o + 1) * P],
                                xn[:, c, :],
                                start=(c == 0), stop=(c == CCH - 1),
                            )
                        nc.vector.tensor_scalar(
                            out=k_sb[:, o, jt * JT:(jt + 1) * JT], in0=pk,
                            scalar1=bk_t[:, o:o + 1], scalar2=None,
                            op0=ALU.add,
                        )
                    for js in range(JT // P):
                        pv = ps.tile([P, C], f32, tag="ps")
                        for c in range(CCH):
                            nc.tensor.matmul(
                                pv,
                                xn[:, c, js * P:(js + 1) * P],
                                wvT[:, c, :],
                                start=(c == 0), stop=(c == CCH - 1),
                            )
                        jc = jt * (JT // P) + js
                        nc.vector.tensor_copy(out=vT_sb[:, jc, :], in_=pv)
                    if jt < IH // JT:
                        it = jt
                        # query i-tile: q matmuls + residual store share xn
                        nc.sync.dma_start(
                            out=xr_dram[:, :, it * JT:(it + 1) * JT].rearrange(
                                "c p i -> p c i"
                            ),
                            in_=xn,
                        )
                        for o in range(CCH):
                            pq = ps.tile([P, JT], f32, tag="ps")
                            for c in range(CCH):
                                nc.tensor.matmul(
                                    pq,
                                    wqT[:, c, o * P:(o + 1) * P],
                                    xn[:, c, :],
                                    start=(c == 0), stop=(c == CCH - 1),
                                )
                            qt = p2.tile([P, JT], f32r, tag="qt")
                            nc.vector.tensor_scalar(
                                out=qt, in0=pq,
                                scalar1=bq_t[:, o:o + 1], scalar2=None,
                                op0=ALU.add,
                            )
                            nc.sync.dma_start(
                                out=q_dram[o, :, it * JT:(it + 1) * JT],
                                in_=qt,
                            )
                        if it == 0:
                            qt2 = pstream.tile(
                                [P, CCH, ITILE], f32r, tag="qt2"
                            )
                            nc.sync.dma_start(out=qt2, in_=q4[:, :, 0:ITILE])
                            xr0 = pstream.tile(
                                [P, CCH, ITILE], f32r, tag="xr", bufs=1
                            )
                            nc.sync.dma_start(out=xr0, in_=xr4[:, :, 0:ITILE])
                            prefetched[0] = (qt2, xr0)

            # ================= Phase 2: attention + proj =================
            with tc.tile_pool(name="p3", bufs=2) as p3:
                wpT = wpool.tile([P, CCH, C], f32r, tag="w")
                nc.sync.dma_start(
                    out=wpT, in_=wpT_h[:, :].rearrange("(c p) o -> p c o", p=P)
                )
                # v-bias folds to a constant output bias: y += wp@bv + bp
                # (attention rows sum to 1 after the linv scaling).
                bias2 = pstream.tile([P, CCH], f32, tag="bias2", bufs=1)
                for oc in range(CCH):
                    pbv = ps.tile([P, 1], f32, tag="ps")
                    for cc in range(CCH):
                        nc.tensor.matmul(
                            pbv,
                            wpT[:, cc, oc * P:(oc + 1) * P].bitcast(f32),
                            bv_t[:, cc:cc + 1],
                            start=(cc == 0), stop=(cc == CCH - 1),
                        )
                    nc.vector.tensor_scalar(
                        out=bias2[:, oc:oc + 1], in0=pbv,
                        scalar1=bp_t[:, oc:oc + 1], scalar2=None, op0=ALU.add,
                    )
                for t in range(NIT):
                    isl = slice(t * ITILE, (t + 1) * ITILE)
                    if t in prefetched:
                        qt2, xr = prefetched[t]
                    else:
                        qt2 = pstream.tile([P, CCH, ITILE], f32r, tag="qt2")
                        nc.sync.dma_start(out=qt2, in_=q4[:, :, isl])
                        xr = pstream.tile([P, CCH, ITILE], f32r, tag="xr", bufs=1)
                        nc.sync.dma_start(out=xr, in_=xr4[:, :, isl])
                    PT = p3.tile([P, NJC, ITILE], f32r, tag="PT", bufs=1)
                    # two alternating partial softmax-denominator
                    # accumulators: a single serial 32-add DVE chain would lag
                    # the exps and stall PE at the pl matmul.
                    lp4 = p3.tile([P, 2, ITILE], f32r, tag="lp4", bufs=1)
                    for jc in range(NJC):
                        pS = ps.tile([P, ITILE], f32, tag="ps")
                        for c in range(CCH):
                            nc.tensor.matmul(
                                pS,
                                k_sb[:, c, jc * P:(jc + 1) * P],
                                qt2[:, c, :],
                                start=(c == 0), stop=(c == CCH - 1),
                            )
                        nc.scalar.activation(
                            out=PT[:, jc, :], in_=pS, func=ACT.Exp,
                            scale=ATT_SCALE,
                        )
                        acc = lp4[:, jc % 2, :]
                        if jc < 2:
                            nc.vector.tensor_copy(out=acc, in_=PT[:, jc, :])
                        else:
                            nc.vector.tensor_add(acc, acc, PT[:, jc, :])

                    # PV before the l-reduction matmuls: PE stays busy while
                    # DVE finishes the partial sums.
                    ao = p3.tile([P, CCH, ITILE], f32r, tag="ao", bufs=1)
                    for cc in range(CCH):
                        pPV = ps.tile([P, ITILE], f32, tag="ps")
                        for jc in range(NJC):
                            nc.tensor.matmul(
                                pPV,
                                vT_sb[:, jc, cc * P:(cc + 1) * P],
                                PT[:, jc, :],
                                start=(jc == 0), stop=(jc == NJC - 1),
                            )
                        nc.vector.tensor_copy(out=ao[:, cc, :], in_=pPV)

                    nc.vector.tensor_add(lp4[:, 0, :], lp4[:, 0, :], lp4[:, 1, :])
                    pl = ps.tile([1, ITILE], f32, tag="ps")
                    nc.tensor.matmul(
                        pl, ones_col_r, lp4[:, 0, :],
                        start=True, stop=True,
                    )
                    linv = pstream.tile([1, ITILE], f32r, tag="linv", bufs=1)
                    nc.vector.reciprocal(out=linv, in_=pl)
                    pb = ps.tile([P, ITILE], f32, tag="ps")
                    nc.tensor.matmul(
                        pb, ones_row_r, linv,
                        start=True, stop=True,
                    )
                    lb = p3.tile([P, ITILE], f32, tag="lb", bufs=1)
                    nc.vector.tensor_copy(out=lb, in_=pb)
                    for oc in range(CCH):
                        pY = ps.tile([P, ITILE], f32, tag="ps")
                        for cc in range(CCH):
                            nc.tensor.matmul(
                                pY,
                                wpT[:, cc, oc * P:(oc + 1) * P],
                                ao[:, cc, :],
                                start=(cc == 0), stop=(cc == CCH - 1),
                            )
                        yt = p3.tile([P, ITILE], f32, tag="yt")
                        nc.vector.tensor_mul(yt, pY, lb)
                        nc.vector.tensor_scalar(
                            out=yt, in0=yt, scalar1=bias2[:, oc:oc + 1],
                            scalar2=None, op0=ALU.add,
                        )
                        nc.vector.tensor_add(yt, yt, xr[:, oc, :])
                        nc.sync.dma_start(out=y3[:, oc, isl], in_=yt)
    nc.finalize()
    return nc


def _make_in_maps(x, gn_gamma, gn_beta, wq, bq, wk, bk, wv, bv, wp, bp):
    x = np.asarray(x, dtype=np.float32)
    xr = np.ascontiguousarray(x.reshape(B, C, N))
    wqT = np.ascontiguousarray(np.asarray(wq, np.float32).T)
    wkT = np.ascontiguousarray(np.asarray(wk, np.float32).T)
    wvT = np.ascontiguousarray(np.asarray(wv, np.float32).T)
    wpT = np.ascontiguousarray(np.asarray(wp, np.float32).T)
    shared = {
        "wqT": wqT, "wkT": wkT, "wvT": wvT, "wpT": wpT,
        "gamma": np.ascontiguousarray(np.asarray(gn_gamma, np.float32)),
        "beta": np.ascontiguousarray(np.asarray(gn_beta, np.float32)),
        "bq": np.ascontiguousarray(np.asarray(bq, np.float32)),
        "bk": np.ascontiguousarray(np.asarray(bk, np.float32)),
        "bv": np.ascontiguousarray(np.asarray(bv, np.float32)),
        "bp": np.ascontiguousarray(np.asarray(bp, np.float32)),
    }
    in_maps = []
    for core in range(8):
        b, ih = core // 2, core % 2
        # rotate spatial columns so this core's query half is always 0..IH-1
        # (GroupNorm and attention are permutation-invariant over positions)
        xrot = xr[b] if ih == 0 else np.concatenate(
            [xr[b][:, IH:], xr[b][:, :IH]], axis=1
        )
        in_maps.append({"x": np.ascontiguousarray(xrot), **shared})

    return in_maps


def _gather(results):
    out = np.empty((B, C, N), np.float32)
    for core in range(8):
        b, ih = core // 2, core % 2
        out[b][:, ih * IH:(ih + 1) * IH] = results[core]["y"]
    return out.reshape(B, C, 64, 64)


def kernel(**inputs):
    global LAST_EXEC_NS
    from concourse.bass_utils import run_bass_kernel_spmd

    if "nc" not in _CACHE:
        _CACHE["nc"] = _build_nc()
    nc = _CACHE["nc"]
    in_maps = _make_in_maps(**inputs)
    res = run_bass_kernel_spmd(nc, in_maps, list(range(8)))
    LAST_EXEC_NS = res.exec_time_ns
    return _gather(res.results)

